# revision 20
# baseline (speedup 1.0000x reference)
"""Additive attention (Bahdanau) fused Trainium2 kernel, data-parallel over batch.

Math: with q = Q @ Wq.T + bq, k = K @ Wk.T + bk,
  scores[b,i,j] = tanh( w_s . (q[b,i] + k[b,j]) + b_s )
                = tanh( qs[b,i] + ks[b,j] + c )
where qs = Q @ (Wq.T @ w_s), ks = K @ (Wk.T @ w_s), c = (bq+bk).w_s + b_s.
The (B,Lq,Lk,H) intermediate is never materialized. tanh bounds scores in
[-1,1], so the softmax needs no max-subtraction; masking is a per-key -120
additive bias on the exp input (exp(-120±1) underflows to exactly 0, matching
the reference's -1e6 fill). The softmax denominator comes from a ones-column
appended to V inside the attn @ V matmul (PSUM accumulates fp32).

Layout on each core (batch element b): scores kept TRANSPOSED [k, q] so the
key dim sits on partitions: the attn @ V contraction over k runs on the PE,
the softmax mask is a per-partition exp bias, and no 512x512 transpose is
ever needed. qs enters via a PE ones-broadcast, ks via the ACT bias port.

Sharding: batch B=8 across 8 NeuronCores, one batch element per core. Inputs
are host-packed into contiguous [128, N] blocks (pure relayout) so each lands
with one simple 2D DMA split over two HWDGE queues (sync + scalar).
"""

from contextlib import ExitStack

import numpy as np

import concourse.tile as tile
from concourse import bacc, mybir
from concourse.bass import ts
from concourse.bass_utils import run_bass_kernel_spmd
from concourse.masks import make_identity

B, LQ, LK = 8, 512, 512
F = 256          # feature dim of Q/K/V
H = 128          # hidden dim of the additive-attention MLP
P = 128          # SBUF partitions
QT = LQ // P     # query chunks per core
KT = LK // P     # key chunks per core
NCORES = 8
MASK_BIAS = -120.0  # exp(-120 + [-1,1]) == 0.0 in fp32

F32 = mybir.dt.float32
BF16 = mybir.dt.bfloat16

# wpack column layout: Wq | Wk | ws_bcast(128) | ws | bq | bk | vl | iota4 | bs
WQ0, WK0 = 0, F
WSB0 = 2 * F                      # ws broadcast block [128h, 128] (col-replicated)
WS_C = WSB0 + P                   # ws as a single column
BQ_C, BK_C, VL_C = WS_C + 1, WS_C + 2, WS_C + 3
IOTA0 = WS_C + 4
BS_C = IOTA0 + KT
WPACK_W = BS_C + 1

TRACE = False
LAST_RESULT = None


def _emit(tc, d):
    nc = tc.nc
    X = mybir.AxisListType
    A = mybir.AluOpType
    AF = mybir.ActivationFunctionType

    with ExitStack() as ctx:
        consts = ctx.enter_context(tc.tile_pool(name="consts", bufs=1))
        big = ctx.enter_context(tc.tile_pool(name="big", bufs=1))
        small = ctx.enter_context(tc.tile_pool(name="small", bufs=1))
        st_pool = ctx.enter_context(tc.tile_pool(name="st", bufs=2))
        ps_uv = ctx.enter_context(tc.tile_pool(name="ps_uv", bufs=1, space="PSUM"))
        ps_qst = ctx.enter_context(tc.tile_pool(name="ps_qst", bufs=1, space="PSUM"))
        ps_c = ctx.enter_context(tc.tile_pool(name="ps_c", bufs=1, space="PSUM"))
        ps_bc = ctx.enter_context(tc.tile_pool(name="ps_bc", bufs=1, space="PSUM"))
        ps_acc = ctx.enter_context(tc.tile_pool(name="ps_acc", bufs=1, space="PSUM"))

        # ---- DMA issue: sync [wpack, kpk, out], scalar [qpk, vaug] ----
        # wpack first on sync: it gates the whole setup chain
        wpack = consts.tile([P, WPACK_W], F32)
        nc.sync.dma_start(wpack, d["wpack"])
        q_sb = big.tile([P, QT * F], F32)
        nc.scalar.dma_start(q_sb, d["qpk"])
        k_sb = big.tile([P, KT * F], F32)
        nc.sync.dma_start(k_sb, d["kpk"])
        aug_f = big.tile([P, KT * (F + 1)], F32)
        nc.scalar.dma_start(aug_f, d["vaug"])

        # ACT table prefetch: dummy Exp loads exp_and_others during the DMAs
        warm = consts.tile([1, 1], F32)
        nc.vector.memset(warm, 0.0)
        nc.scalar.activation(warm, warm, AF.Exp)

        ones_row = consts.tile([1, P], BF16)
        nc.vector.memset(ones_row, 1.0)
        id128 = consts.tile([P, P], F32)
        make_identity(nc, id128)

        # ---- weight prep (all off the q/k critical path) ----
        # bf16 view of [Wq | Wk | ws_bcast]; one matmul gives [u|v] broadcast
        # to all partitions: uv_bc[p, f] = sum_h ws[h] * W[h, f]
        wf_bf = consts.tile([P, WSB0 + P], BF16)
        nc.vector.tensor_copy(wf_bf, wpack[:, 0:WSB0 + P])
        uv_bc_ps = ps_uv.tile([P, 2 * F], F32)
        nc.tensor.matmul(uv_bc_ps, wf_bf[:, WSB0:WSB0 + P], wf_bf[:, 0:2 * F],
                         start=True, stop=True)

        # expbias[p, c] = (p + 128c >= valid_len) ? MASK_BIAS : 0
        expbias = small.tile([P, KT], F32)
        nc.vector.tensor_scalar(expbias, wpack[:, IOTA0:IOTA0 + KT],
                                wpack[:, VL_C:VL_C + 1], MASK_BIAS, A.is_ge, A.mult)

        # c = ws.(bq+bk) + bs, broadcast to a [P, 1] column (folds into ks)
        bsum = small.tile([H, 1], F32)
        nc.vector.tensor_tensor(bsum, wpack[:, BQ_C:BQ_C + 1],
                                wpack[:, BK_C:BK_C + 1], A.add)
        c_ps = ps_c.tile([1, 1], F32)
        nc.tensor.matmul(c_ps, wpack[:, WS_C:WS_C + 1], bsum, start=True, stop=True)
        c_sb = small.tile([1, 1], F32)
        nc.vector.tensor_tensor(c_sb, c_ps, wpack[0:1, BS_C:BS_C + 1], A.add)
        ones_f32 = consts.tile([1, P], F32)
        nc.vector.memset(ones_f32, 1.0)
        cbc_ps = ps_c.tile([P, 1], F32, tag="c_ps")
        nc.tensor.matmul(cbc_ps, ones_f32, c_sb, start=True, stop=True)
        c_bc = small.tile([P, 1], F32)
        nc.vector.tensor_copy(c_bc, cbc_ps)

        # v half of uv_bc to SBUF first (GpSimd cannot read PSUM)
        v_bc = small.tile([P, F], F32)
        nc.vector.tensor_copy(v_bc, uv_bc_ps[:, F:2 * F])

        # ---- qs matvec on DVE; PE transpose per chunk into one [1, LQ] row ----
        qs_pack = small.tile([P, QT], F32)
        qsT_ps = ps_qst.tile([1, LQ], F32)
        for t in range(QT):
            qm = st_pool.tile([P, F], BF16, tag="qm", bufs=4)
            nc.vector.tensor_tensor(qm, q_sb[:, ts(t, F)], uv_bc_ps[:, 0:F], A.mult)
            nc.vector.reduce_sum(qs_pack[:, t:t + 1], qm, axis=X.X)
            nc.tensor.matmul(qsT_ps[0:1, ts(t, P)], qs_pack[:, t:t + 1], id128,
                             start=True, stop=True)
        qs_row = small.tile([1, LQ], BF16)
        nc.vector.tensor_copy(qs_row, qsT_ps)

        # ks matvec: multiplies on GpSimd, one batched reduce + one batched
        # +c fold on DVE (ks is not on the critical path; qs is)
        km_all = big.tile([P, KT, F], BF16)
        for t in range(KT):
            nc.gpsimd.tensor_tensor(km_all[:, t, :], k_sb[:, ts(t, F)], v_bc,
                                    A.mult)
        ks_raw = small.tile([P, KT], F32)
        nc.vector.reduce_sum(ks_raw, km_all, axis=X.X)
        ks_pack = small.tile([P, KT], F32)
        nc.vector.tensor_scalar(ks_pack, ks_raw, c_bc, None, A.add)

        # aug -> bf16 per chunk on GpSimd (idle otherwise)
        aug_bf = big.tile([P, KT * (F + 1)], BF16)
        for c in range(KT):
            nc.gpsimd.tensor_copy(aug_bf[:, c * (F + 1):(c + 1) * (F + 1)],
                                  aug_f[:, c * (F + 1):(c + 1) * (F + 1)])

        # broadcast qs+c to all partitions: [P, LQ] fp32 in PSUM
        qs_bc_ps = ps_bc.tile([P, LQ], F32)
        nc.tensor.matmul(qs_bc_ps, ones_row, qs_row, start=True, stop=True)

        # ---- scores.T -> exp (bf16) -> attn.T @ [V | 1] ----
        accs = [ps_acc.tile([P, F + 1], F32, tag=f"acc{qc}", name=f"acc{qc}")
                for qc in range(QT)]
        for c in range(KT):
            sT = st_pool.tile([P, LQ], F32, tag="sT")
            nc.scalar.activation(sT, qs_bc_ps, AF.Tanh, bias=ks_pack[:, c:c + 1])
            eT = st_pool.tile([P, LQ], BF16, tag="eT")
            nc.scalar.activation(eT, sT, AF.Exp, bias=expbias[:, c:c + 1])
            for qc in range(QT):
                nc.tensor.matmul(accs[qc], eT[:, ts(qc, P)],
                                 aug_bf[:, c * (F + 1):(c + 1) * (F + 1)],
                                 start=(c == 0), stop=(c == KT - 1))

        # ---- normalize (split ACT/DVE), store in halves ----
        out_sb = big.tile([P, QT * F], F32)
        for qc in range(QT):
            rec = small.tile([P, 1], F32, tag=f"rec{qc}", name=f"rec{qc}")
            nc.vector.reciprocal(rec, accs[qc][:, F:F + 1])
            if qc % 2 == 0:
                nc.scalar.activation(out_sb[:, ts(qc, F)], accs[qc][:, 0:F],
                                     AF.Copy, bias=0.0, scale=rec)
            else:
                nc.vector.tensor_scalar(out_sb[:, ts(qc, F)], accs[qc][:, 0:F],
                                        rec, None, A.mult)
            if qc == 1:
                nc.sync.dma_start(d["out"][:, 0:2 * F], out_sb[:, 0:2 * F])
        nc.sync.dma_start(d["out"][:, 2 * F:4 * F], out_sb[:, 2 * F:4 * F])


_NC = None


def _build_nc():
    nc = bacc.Bacc("TRN2", target_bir_lowering=False, debug=False, num_devices=1)
    d = {}
    d["wpack"] = nc.dram_tensor("wpack", [P, WPACK_W], F32, kind="ExternalInput").ap()
    d["qpk"] = nc.dram_tensor("qpk", [P, QT * F], F32, kind="ExternalInput").ap()
    d["kpk"] = nc.dram_tensor("kpk", [P, KT * F], F32, kind="ExternalInput").ap()
    d["vaug"] = nc.dram_tensor("vaug", [P, KT * (F + 1)], F32, kind="ExternalInput").ap()
    d["out"] = nc.dram_tensor("out", [P, QT * F], F32, kind="ExternalOutput").ap()

    with tile.TileContext(nc) as tc:
        _emit(tc, d)
    nc.compile()
    return nc


def get_nc():
    global _NC
    if _NC is None:
        _NC = _build_nc()
    return _NC


def make_in_maps(queries, keys, values, valid_lens, Wq, bq, Wk, bk, w_s, b_s):
    f32 = lambda a: np.asarray(a, dtype=np.float32)
    qs, ks, vs = f32(queries), f32(keys), f32(values)
    vl = np.asarray(valid_lens)

    wpack = np.zeros((P, WPACK_W), np.float32)
    wpack[:, WQ0:WQ0 + F] = f32(Wq)
    wpack[:, WK0:WK0 + F] = f32(Wk)
    wpack[:, WSB0:WSB0 + P] = f32(w_s)[:, None]          # ws broadcast block
    wpack[:, WS_C] = f32(w_s)
    wpack[:, BQ_C] = f32(bq)
    wpack[:, BK_C] = f32(bk)
    wpack[:, IOTA0:IOTA0 + KT] = (np.arange(P, dtype=np.float32)[:, None]
                                  + P * np.arange(KT, dtype=np.float32)[None, :])
    wpack[0, BS_C] = f32(b_s).reshape(-1)[0]

    # [L, F] -> [P, T*F] with col t*F+f <-> row t*P+p  (pure relayout)
    def pack_tiles(a):  # a: [L, F]
        t = a.reshape(-1, P, F)                    # [T, P, F]
        return np.ascontiguousarray(t.transpose(1, 0, 2).reshape(P, -1))

    in_maps = []
    for b in range(NCORES):
        m = {}
        wp = wpack.copy()
        wp[:, VL_C] = float(vl[b])
        m["wpack"] = wp
        m["qpk"] = pack_tiles(qs[b])
        m["kpk"] = pack_tiles(ks[b])
        va = np.ones((KT, P, F + 1), np.float32)
        va[:, :, :F] = vs[b].reshape(KT, P, F)
        m["vaug"] = np.ascontiguousarray(va.transpose(1, 0, 2).reshape(P, -1))
        in_maps.append(m)
    return in_maps


def kernel(queries, keys, values, valid_lens, Wq, bq, Wk, bk, w_s, b_s):
    global LAST_RESULT
    nc = get_nc()
    in_maps = make_in_maps(queries, keys, values, valid_lens, Wq, bq, Wk, bk, w_s, b_s)
    res = run_bass_kernel_spmd(nc, in_maps, list(range(NCORES)), trace=TRACE)
    LAST_RESULT = res
    out = np.stack([res.results[b]["out"] for b in range(NCORES)], axis=0)
    # [P, T*F] -> [LQ, F]
    out = out.reshape(B, P, QT, F).transpose(0, 2, 1, 3).reshape(B, LQ, F)
    return np.ascontiguousarray(out)


# revision 23
# speedup vs baseline: 1.0183x; 1.0183x over previous
"""Additive attention (Bahdanau) fused Trainium2 kernel, data-parallel over batch.

Math: with q = Q @ Wq.T + bq, k = K @ Wk.T + bk,
  scores[b,i,j] = tanh( w_s . (q[b,i] + k[b,j]) + b_s )
                = tanh( qs[b,i] + ks[b,j] + c )
where qs = Q @ (Wq.T @ w_s), ks = K @ (Wk.T @ w_s), c = (bq+bk).w_s + b_s.
The (B,Lq,Lk,H) intermediate is never materialized. tanh bounds scores in
[-1,1], so the softmax needs no max-subtraction; masking is a per-key -120
additive bias on the exp input (exp(-120±1) underflows to exactly 0, matching
the reference's -1e6 fill). The softmax denominator comes from a ones-column
appended to V inside the attn @ V matmul (PSUM accumulates fp32).

Layout on each core (batch element b): scores kept TRANSPOSED [k, q] so the
key dim sits on partitions: the attn @ V contraction over k runs on the PE,
the softmax mask is a per-partition exp bias, and no 512x512 transpose is
ever needed. qs enters via a PE ones-broadcast, ks via the ACT bias port.

Sharding: batch B=8 across 8 NeuronCores, one batch element per core. Inputs
are host-packed into contiguous [128, N] blocks (pure relayout) so each lands
with one simple 2D DMA split over two HWDGE queues (sync + scalar).
"""

from contextlib import ExitStack

import numpy as np

import concourse.tile as tile
from concourse import bacc, mybir
from concourse.bass import ts
from concourse.bass_utils import run_bass_kernel_spmd
from concourse.masks import make_identity

B, LQ, LK = 8, 512, 512
F = 256          # feature dim of Q/K/V
H = 128          # hidden dim of the additive-attention MLP
P = 128          # SBUF partitions
QT = LQ // P     # query chunks per core
KT = LK // P     # key chunks per core
NCORES = 8
MASK_BIAS = -120.0  # exp(-120 + [-1,1]) == 0.0 in fp32

F32 = mybir.dt.float32
BF16 = mybir.dt.bfloat16

# wpack column layout: Wq | Wk | ws_bcast(128) | ws | bq | bk | vl | iota4 | bs
WQ0, WK0 = 0, F
WSB0 = 2 * F                      # ws broadcast block [128h, 128] (col-replicated)
WS_C = WSB0 + P                   # ws as a single column
BQ_C, BK_C, VL_C = WS_C + 1, WS_C + 2, WS_C + 3
IOTA0 = WS_C + 4
BS_C = IOTA0 + KT
WPACK_W = BS_C + 1

TRACE = False
LAST_RESULT = None


def _emit(tc, d):
    nc = tc.nc
    X = mybir.AxisListType
    A = mybir.AluOpType
    AF = mybir.ActivationFunctionType

    with ExitStack() as ctx:
        consts = ctx.enter_context(tc.tile_pool(name="consts", bufs=1))
        big = ctx.enter_context(tc.tile_pool(name="big", bufs=1))
        small = ctx.enter_context(tc.tile_pool(name="small", bufs=1))
        st_pool = ctx.enter_context(tc.tile_pool(name="st", bufs=2))
        ps_uv = ctx.enter_context(tc.tile_pool(name="ps_uv", bufs=1, space="PSUM"))
        ps_qst = ctx.enter_context(tc.tile_pool(name="ps_qst", bufs=1, space="PSUM"))
        ps_c = ctx.enter_context(tc.tile_pool(name="ps_c", bufs=1, space="PSUM"))
        ps_bc = ctx.enter_context(tc.tile_pool(name="ps_bc", bufs=1, space="PSUM"))
        ps_acc = ctx.enter_context(tc.tile_pool(name="ps_acc", bufs=1, space="PSUM"))

        # ---- DMA issue: sync [wpack, kpk, out], scalar [qpk, vaug] ----
        # wpack first on sync: it gates the whole setup chain
        wpack = consts.tile([P, WPACK_W], F32)
        nc.sync.dma_start(wpack, d["wpack"])
        q_sb = big.tile([P, QT * F], F32)
        nc.scalar.dma_start(q_sb.rearrange("p (t f) -> p t f", f=F),
                            d["qpk"].rearrange("p (t f) -> p t f", f=F))
        k_sb = big.tile([P, KT * F], F32)
        nc.sync.dma_start(k_sb.rearrange("p (t f) -> p t f", f=F),
                          d["kpk"].rearrange("p (t f) -> p t f", f=F))
        aug_f = big.tile([P, KT * (F + 1)], F32)
        nc.scalar.dma_start(aug_f.rearrange("p (t g) -> p t g", g=F + 1),
                            d["vaug"].rearrange("p (t g) -> p t g", g=F + 1))

        # ACT table prefetch: dummy Exp loads exp_and_others during the DMAs
        warm = consts.tile([1, 1], F32)
        nc.vector.memset(warm, 0.0)
        nc.scalar.activation(warm, warm, AF.Exp)

        ones_row = consts.tile([1, P], BF16)
        nc.vector.memset(ones_row, 1.0)
        id128 = consts.tile([P, P], F32)
        make_identity(nc, id128)

        # ---- weight prep (all off the q/k critical path) ----
        # One matmul gives [u|v] broadcast to all partitions:
        # uv_bc[p, f] = sum_h ws[h] * W[h, f]
        uv_bc_ps = ps_uv.tile([P, 2 * F], F32)
        nc.tensor.matmul(uv_bc_ps, wpack[:, WSB0:WSB0 + P], wpack[:, 0:2 * F],
                         start=True, stop=True)

        # expbias[p, c] = (p + 128c >= valid_len) ? MASK_BIAS : 0
        expbias = small.tile([P, KT], F32)
        nc.vector.tensor_scalar(expbias, wpack[:, IOTA0:IOTA0 + KT],
                                wpack[:, VL_C:VL_C + 1], MASK_BIAS, A.is_ge, A.mult)

        # c = ws.(bq+bk) + bs, broadcast to a [P, 1] column (folds into ks)
        bsum = small.tile([H, 1], F32)
        nc.vector.tensor_tensor(bsum, wpack[:, BQ_C:BQ_C + 1],
                                wpack[:, BK_C:BK_C + 1], A.add)
        c_ps = ps_c.tile([1, 1], F32)
        nc.tensor.matmul(c_ps, wpack[:, WS_C:WS_C + 1], bsum, start=True, stop=True)
        c_sb = small.tile([1, 1], F32)
        nc.vector.tensor_tensor(c_sb, c_ps, wpack[0:1, BS_C:BS_C + 1], A.add)
        ones_f32 = consts.tile([1, P], F32)
        nc.vector.memset(ones_f32, 1.0)
        cbc_ps = ps_c.tile([P, 1], F32, tag="c_ps")
        nc.tensor.matmul(cbc_ps, ones_f32, c_sb, start=True, stop=True)
        c_bc = small.tile([P, 1], F32)
        nc.vector.tensor_copy(c_bc, cbc_ps)

        # v half of uv_bc to SBUF first (GpSimd cannot read PSUM)
        v_bc = small.tile([P, F], F32)
        nc.vector.tensor_copy(v_bc, uv_bc_ps[:, F:2 * F])

        # ---- qs matvec on DVE; PE transpose per chunk into one [1, LQ] row ----
        qs_pack = small.tile([P, QT], F32)
        qsT_ps = ps_qst.tile([1, LQ], F32)
        for t in range(QT):
            qm = st_pool.tile([P, F], BF16, tag="qm", bufs=4)
            nc.vector.tensor_tensor(qm, q_sb[:, ts(t, F)], uv_bc_ps[:, 0:F], A.mult)
            nc.vector.reduce_sum(qs_pack[:, t:t + 1], qm, axis=X.X)
            nc.tensor.matmul(qsT_ps[0:1, ts(t, P)], qs_pack[:, t:t + 1], id128,
                             start=True, stop=True)
        qs_row = small.tile([1, LQ], BF16)
        nc.vector.tensor_copy(qs_row, qsT_ps)

        # ks matvec: multiplies on GpSimd, per-chunk reduces on DVE, then one
        # batched +c fold (ks is not on the critical path; qs is)
        ks_raw = small.tile([P, KT], F32)
        for t in range(KT):
            km = st_pool.tile([P, F], BF16, tag="km", bufs=4)
            nc.gpsimd.tensor_tensor(km, k_sb[:, ts(t, F)], v_bc, A.mult)
            nc.vector.reduce_sum(ks_raw[:, t:t + 1], km, axis=X.X)
        ks_pack = small.tile([P, KT], F32)
        nc.vector.tensor_scalar(ks_pack, ks_raw, c_bc, None, A.add)

        # aug -> bf16 per chunk on GpSimd (idle otherwise)
        aug_bf = big.tile([P, KT * (F + 1)], BF16)
        for c in range(KT):
            nc.gpsimd.tensor_copy(aug_bf[:, c * (F + 1):(c + 1) * (F + 1)],
                                  aug_f[:, c * (F + 1):(c + 1) * (F + 1)])

        # broadcast qs+c to all partitions: [P, LQ] fp32 in PSUM
        qs_bc_ps = ps_bc.tile([P, LQ], F32)
        nc.tensor.matmul(qs_bc_ps, ones_row, qs_row, start=True, stop=True)

        # ---- scores.T -> exp (bf16) -> attn.T @ [V | 1] ----
        accs = [ps_acc.tile([P, F + 1], F32, tag=f"acc{qc}", name=f"acc{qc}")
                for qc in range(QT)]
        for c in range(KT):
            sT = st_pool.tile([P, LQ], F32, tag="sT")
            nc.scalar.activation(sT, qs_bc_ps, AF.Tanh, bias=ks_pack[:, c:c + 1])
            eT = st_pool.tile([P, LQ], BF16, tag="eT")
            nc.scalar.activation(eT, sT, AF.Exp, bias=expbias[:, c:c + 1])
            for qc in range(QT):
                nc.tensor.matmul(accs[qc], eT[:, ts(qc, P)],
                                 aug_bf[:, c * (F + 1):(c + 1) * (F + 1)],
                                 start=(c == 0), stop=(c == KT - 1))

        # ---- normalize (split ACT/DVE), store in halves ----
        out_sb = big.tile([P, QT * F], F32)
        for qc in range(QT):
            rec = small.tile([P, 1], F32, tag=f"rec{qc}", name=f"rec{qc}")
            nc.vector.reciprocal(rec, accs[qc][:, F:F + 1])
            if qc % 2 == 0:
                nc.scalar.activation(out_sb[:, ts(qc, F)], accs[qc][:, 0:F],
                                     AF.Copy, bias=0.0, scale=rec)
            else:
                nc.vector.tensor_scalar(out_sb[:, ts(qc, F)], accs[qc][:, 0:F],
                                        rec, None, A.mult)
            if qc == 1:
                nc.sync.dma_start(d["out"][:, 0:2 * F], out_sb[:, 0:2 * F])
        nc.sync.dma_start(d["out"][:, 2 * F:4 * F], out_sb[:, 2 * F:4 * F])


_NC = None


def _build_nc():
    nc = bacc.Bacc("TRN2", target_bir_lowering=False, debug=False, num_devices=1)
    d = {}
    d["wpack"] = nc.dram_tensor("wpack", [P, WPACK_W], F32, kind="ExternalInput").ap()
    d["qpk"] = nc.dram_tensor("qpk", [P, QT * F], F32, kind="ExternalInput").ap()
    d["kpk"] = nc.dram_tensor("kpk", [P, KT * F], F32, kind="ExternalInput").ap()
    d["vaug"] = nc.dram_tensor("vaug", [P, KT * (F + 1)], F32, kind="ExternalInput").ap()
    d["out"] = nc.dram_tensor("out", [P, QT * F], F32, kind="ExternalOutput").ap()

    with tile.TileContext(nc) as tc:
        _emit(tc, d)
    nc.compile()
    return nc


def get_nc():
    global _NC
    if _NC is None:
        _NC = _build_nc()
    return _NC


def make_in_maps(queries, keys, values, valid_lens, Wq, bq, Wk, bk, w_s, b_s):
    f32 = lambda a: np.asarray(a, dtype=np.float32)
    qs, ks, vs = f32(queries), f32(keys), f32(values)
    vl = np.asarray(valid_lens)

    wpack = np.zeros((P, WPACK_W), np.float32)
    wpack[:, WQ0:WQ0 + F] = f32(Wq)
    wpack[:, WK0:WK0 + F] = f32(Wk)
    wpack[:, WSB0:WSB0 + P] = f32(w_s)[:, None]          # ws broadcast block
    wpack[:, WS_C] = f32(w_s)
    wpack[:, BQ_C] = f32(bq)
    wpack[:, BK_C] = f32(bk)
    wpack[:, IOTA0:IOTA0 + KT] = (np.arange(P, dtype=np.float32)[:, None]
                                  + P * np.arange(KT, dtype=np.float32)[None, :])
    wpack[0, BS_C] = f32(b_s).reshape(-1)[0]

    # [L, F] -> [P, T*F] with col t*F+f <-> row t*P+p  (pure relayout)
    def pack_tiles(a):  # a: [L, F]
        t = a.reshape(-1, P, F)                    # [T, P, F]
        return np.ascontiguousarray(t.transpose(1, 0, 2).reshape(P, -1))

    in_maps = []
    for b in range(NCORES):
        m = {}
        wp = wpack.copy()
        wp[:, VL_C] = float(vl[b])
        m["wpack"] = wp
        m["qpk"] = pack_tiles(qs[b])
        m["kpk"] = pack_tiles(ks[b])
        va = np.ones((KT, P, F + 1), np.float32)
        va[:, :, :F] = vs[b].reshape(KT, P, F)
        m["vaug"] = np.ascontiguousarray(va.transpose(1, 0, 2).reshape(P, -1))
        in_maps.append(m)
    return in_maps


def kernel(queries, keys, values, valid_lens, Wq, bq, Wk, bk, w_s, b_s):
    global LAST_RESULT
    nc = get_nc()
    in_maps = make_in_maps(queries, keys, values, valid_lens, Wq, bq, Wk, bk, w_s, b_s)
    res = run_bass_kernel_spmd(nc, in_maps, list(range(NCORES)), trace=TRACE)
    LAST_RESULT = res
    out = np.stack([res.results[b]["out"] for b in range(NCORES)], axis=0)
    # [P, T*F] -> [LQ, F]
    out = out.reshape(B, P, QT, F).transpose(0, 2, 1, 3).reshape(B, LQ, F)
    return np.ascontiguousarray(out)


# revision 29
# speedup vs baseline: 1.0944x; 1.0747x over previous
"""Additive attention (Bahdanau) fused Trainium2 kernel, data-parallel over batch.

Math: with q = Q @ Wq.T + bq, k = K @ Wk.T + bk,
  scores[b,i,j] = tanh( w_s . (q[b,i] + k[b,j]) + b_s )
                = tanh( qs[b,i] + ks[b,j] + c )
where qs = Q @ (Wq.T @ w_s), ks = K @ (Wk.T @ w_s), c = (bq+bk).w_s + b_s.
The (B,Lq,Lk,H) intermediate is never materialized. tanh bounds scores in
[-1,1], so the softmax needs no max-subtraction; masking is a per-key -120
additive bias on the exp input (exp(-120±1) underflows to exactly 0, matching
the reference's -1e6 fill). The softmax denominator comes from a ones-column
appended to V inside the attn @ V matmul (PSUM accumulates fp32).

Layout on each core (batch element b): scores kept TRANSPOSED [k, q] so the
key dim sits on partitions: the attn @ V contraction over k runs on the PE,
the softmax mask is a per-partition exp bias, and no 512x512 transpose is
ever needed. qs enters via a PE ones-broadcast, ks via the ACT bias port.

Sharding: batch B=8 across 8 NeuronCores, one batch element per core. Inputs
are host-packed into contiguous [128, N] blocks (pure relayout) so each lands
with one simple 2D DMA split over two HWDGE queues (sync + scalar).
"""

from contextlib import ExitStack

import numpy as np

import concourse.tile as tile
from concourse import bacc, mybir
from concourse.bass import ts
from concourse.bass_utils import run_bass_kernel_spmd
from concourse.masks import make_identity

B, LQ, LK = 8, 512, 512
F = 256          # feature dim of Q/K/V
H = 128          # hidden dim of the additive-attention MLP
P = 128          # SBUF partitions
QT = LQ // P     # query chunks per core
KT = LK // P     # key chunks per core
NCORES = 8
MASK_BIAS = -120.0  # exp(-120 + [-1,1]) == 0.0 in fp32

F32 = mybir.dt.float32
BF16 = mybir.dt.bfloat16

# wpack column layout: Wq | Wk | ws_bcast(128) | ws | bq | bk | vl | iota4 | bs
WQ0, WK0 = 0, F
WSB0 = 2 * F                      # ws broadcast block [128h, 128] (col-replicated)
WS_C = WSB0 + P                   # ws as a single column
BQ_C, BK_C, VL_C = WS_C + 1, WS_C + 2, WS_C + 3
IOTA0 = WS_C + 4
BS_C = IOTA0 + KT
WPACK_W = BS_C + 1

TRACE = False
LAST_RESULT = None


def _emit(tc, d):
    nc = tc.nc
    X = mybir.AxisListType
    A = mybir.AluOpType
    AF = mybir.ActivationFunctionType

    with ExitStack() as ctx:
        consts = ctx.enter_context(tc.tile_pool(name="consts", bufs=1))
        big = ctx.enter_context(tc.tile_pool(name="big", bufs=1))
        small = ctx.enter_context(tc.tile_pool(name="small", bufs=1))
        st_pool = ctx.enter_context(tc.tile_pool(name="st", bufs=2))
        ps_uv = ctx.enter_context(tc.tile_pool(name="ps_uv", bufs=1, space="PSUM"))
        ps_c = ctx.enter_context(tc.tile_pool(name="ps_c", bufs=1, space="PSUM"))
        ps_bc = ctx.enter_context(tc.tile_pool(name="ps_bc", bufs=1, space="PSUM"))
        ps_acc = ctx.enter_context(tc.tile_pool(name="ps_acc", bufs=1, space="PSUM"))

        # ---- DMA issue: sync [wpack, kpk, out], scalar [qpk, vaug] ----
        # wpack first on sync: it gates the whole setup chain
        wpack = consts.tile([P, WPACK_W], F32)
        wdma = nc.sync.dma_start(wpack, d["wpack"])
        q_sb = big.tile([P, QT * F], F32)
        qdma = nc.scalar.dma_start(q_sb.rearrange("p (t f) -> p t f", f=F),
                                   d["qpk"].rearrange("p (t f) -> p t f", f=F))
        k_sb = big.tile([P, KT * F], F32)
        kdma = nc.sync.dma_start(k_sb.rearrange("p (t f) -> p t f", f=F),
                                 d["kpk"].rearrange("p (t f) -> p t f", f=F))
        aug_f = big.tile([P, KT * (F + 1)], F32)
        vdma = nc.scalar.dma_start(aug_f.rearrange("p (t g) -> p t g", g=F + 1),
                                   d["vaug"].rearrange("p (t g) -> p t g", g=F + 1))
        # wpack gates the whole setup chain: packet-level round-robin would
        # otherwise delay its completion to that of the full 1.8MB batch
        for dma in (qdma, kdma, vdma):
            tile.add_dep_helper(dma.ins, wdma.ins, reason="wpack lands first")

        # ACT table prefetch: dummy Exp loads exp_and_others during the DMAs
        warm = consts.tile([1, 1], F32)
        nc.vector.memset(warm, 0.0)
        nc.scalar.activation(warm, warm, AF.Exp)

        id128 = consts.tile([P, P], F32)
        make_identity(nc, id128)

        # ---- weight prep (all off the q/k critical path) ----
        # One matmul gives [u|v] broadcast to all partitions:
        # uv_bc[p, f] = sum_h ws[h] * W[h, f]
        uv_bc_ps = ps_uv.tile([P, 2 * F], F32)
        nc.tensor.matmul(uv_bc_ps, wpack[:, WSB0:WSB0 + P], wpack[:, 0:2 * F],
                         start=True, stop=True)

        # expbias[p, c] = (p + 128c >= valid_len) ? MASK_BIAS : 0
        expbias = small.tile([P, KT], F32)
        nc.vector.tensor_scalar(expbias, wpack[:, IOTA0:IOTA0 + KT],
                                wpack[:, VL_C:VL_C + 1], MASK_BIAS, A.is_ge, A.mult)

        # c = ws.(bq+bk) + bs, broadcast to a [P, 1] column (folds into ks)
        bsum = small.tile([H, 1], F32)
        nc.vector.tensor_tensor(bsum, wpack[:, BQ_C:BQ_C + 1],
                                wpack[:, BK_C:BK_C + 1], A.add)
        c_ps = ps_c.tile([1, 1], F32)
        nc.tensor.matmul(c_ps, wpack[:, WS_C:WS_C + 1], bsum, start=True, stop=True)
        c_sb = small.tile([1, 1], F32)
        nc.vector.tensor_tensor(c_sb, c_ps, wpack[0:1, BS_C:BS_C + 1], A.add)
        ones_f32 = consts.tile([1, P], F32)
        nc.vector.memset(ones_f32, 1.0)
        cbc_ps = ps_c.tile([P, 1], F32, tag="c_ps")
        nc.tensor.matmul(cbc_ps, ones_f32, c_sb, start=True, stop=True)
        c_bc = small.tile([P, 1], F32)
        nc.vector.tensor_copy(c_bc, cbc_ps)

        # v half of uv_bc to SBUF first (GpSimd cannot read PSUM)
        v_bc = small.tile([P, F], F32)
        nc.vector.tensor_copy(v_bc, uv_bc_ps[:, F:2 * F])

        # ---- qs matvec on DVE; PE transpose per chunk into one [1, LQ] row ----
        # Per chunk: qm = q*u (DVE), reduce to qs column (DVE), then ONE
        # matmul fuses transpose+broadcast: lhsT = qs column replicated over
        # the free dim (stride-0), rhs = identity ->
        # out[p, j] = sum_q qs[q] * I[q, j] broadcast to all 128 partitions.
        qs_pack = small.tile([P, QT], F32)
        qs_bc_ps = ps_bc.tile([P, LQ], F32)
        for t in range(QT):
            qm = st_pool.tile([P, F], BF16, tag="qm", bufs=4)
            nc.vector.tensor_tensor(qm, q_sb[:, ts(t, F)], uv_bc_ps[:, 0:F], A.mult)
            nc.vector.reduce_sum(qs_pack[:, t:t + 1], qm, axis=X.X)
            nc.tensor.matmul(qs_bc_ps[:, ts(t, P)],
                             qs_pack[:, t:t + 1].broadcast_to([P, P]), id128,
                             start=True, stop=True)

        # ks matvec: multiplies on GpSimd, per-chunk reduces on DVE, then one
        # batched +c fold (ks is not on the critical path; qs is)
        ks_raw = small.tile([P, KT], F32)
        for t in range(KT):
            km = st_pool.tile([P, F], BF16, tag="km", bufs=4)
            nc.gpsimd.tensor_tensor(km, k_sb[:, ts(t, F)], v_bc, A.mult)
            nc.vector.reduce_sum(ks_raw[:, t:t + 1], km, axis=X.X)
        ks_pack = small.tile([P, KT], F32)
        nc.vector.tensor_scalar(ks_pack, ks_raw, c_bc, None, A.add)

        # aug -> bf16 per chunk on GpSimd (idle otherwise)
        aug_bf = big.tile([P, KT * (F + 1)], BF16)
        for c in range(KT):
            nc.gpsimd.tensor_copy(aug_bf[:, c * (F + 1):(c + 1) * (F + 1)],
                                  aug_f[:, c * (F + 1):(c + 1) * (F + 1)])

        # ---- scores.T -> exp (bf16) -> attn.T @ [V | 1] ----
        accs = [ps_acc.tile([P, F + 1], F32, tag=f"acc{qc}", name=f"acc{qc}")
                for qc in range(QT)]
        for c in range(KT):
            sT = st_pool.tile([P, LQ], F32, tag="sT")
            nc.scalar.activation(sT, qs_bc_ps, AF.Tanh, bias=ks_pack[:, c:c + 1])
            eT = st_pool.tile([P, LQ], BF16, tag="eT")
            nc.scalar.activation(eT, sT, AF.Exp, bias=expbias[:, c:c + 1])
            for qc in range(QT):
                nc.tensor.matmul(accs[qc], eT[:, ts(qc, P)],
                                 aug_bf[:, c * (F + 1):(c + 1) * (F + 1)],
                                 start=(c == 0), stop=(c == KT - 1))

        # ---- normalize (split ACT/DVE), store in halves ----
        out_sb = big.tile([P, QT * F], F32)
        for qc in range(QT):
            rec = small.tile([P, 1], F32, tag=f"rec{qc}", name=f"rec{qc}")
            nc.vector.reciprocal(rec, accs[qc][:, F:F + 1])
            if qc % 2 == 0:
                nc.scalar.activation(out_sb[:, ts(qc, F)], accs[qc][:, 0:F],
                                     AF.Copy, bias=0.0, scale=rec)
            else:
                nc.vector.tensor_scalar(out_sb[:, ts(qc, F)], accs[qc][:, 0:F],
                                        rec, None, A.mult)
            if qc == 1:
                nc.sync.dma_start(d["out"][:, 0:2 * F], out_sb[:, 0:2 * F])
        nc.sync.dma_start(d["out"][:, 2 * F:4 * F], out_sb[:, 2 * F:4 * F])


_NC = None


def _build_nc():
    nc = bacc.Bacc("TRN2", target_bir_lowering=False, debug=False, num_devices=1)
    d = {}
    d["wpack"] = nc.dram_tensor("wpack", [P, WPACK_W], F32, kind="ExternalInput").ap()
    d["qpk"] = nc.dram_tensor("qpk", [P, QT * F], F32, kind="ExternalInput").ap()
    d["kpk"] = nc.dram_tensor("kpk", [P, KT * F], F32, kind="ExternalInput").ap()
    d["vaug"] = nc.dram_tensor("vaug", [P, KT * (F + 1)], F32, kind="ExternalInput").ap()
    d["out"] = nc.dram_tensor("out", [P, QT * F], F32, kind="ExternalOutput").ap()

    with tile.TileContext(nc) as tc:
        _emit(tc, d)
    nc.compile()
    return nc


def get_nc():
    global _NC
    if _NC is None:
        _NC = _build_nc()
    return _NC


def make_in_maps(queries, keys, values, valid_lens, Wq, bq, Wk, bk, w_s, b_s):
    f32 = lambda a: np.asarray(a, dtype=np.float32)
    qs, ks, vs = f32(queries), f32(keys), f32(values)
    vl = np.asarray(valid_lens)

    wpack = np.zeros((P, WPACK_W), np.float32)
    wpack[:, WQ0:WQ0 + F] = f32(Wq)
    wpack[:, WK0:WK0 + F] = f32(Wk)
    wpack[:, WSB0:WSB0 + P] = f32(w_s)[:, None]          # ws broadcast block
    wpack[:, WS_C] = f32(w_s)
    wpack[:, BQ_C] = f32(bq)
    wpack[:, BK_C] = f32(bk)
    wpack[:, IOTA0:IOTA0 + KT] = (np.arange(P, dtype=np.float32)[:, None]
                                  + P * np.arange(KT, dtype=np.float32)[None, :])
    wpack[0, BS_C] = f32(b_s).reshape(-1)[0]

    # [L, F] -> [P, T*F] with col t*F+f <-> row t*P+p  (pure relayout)
    def pack_tiles(a):  # a: [L, F]
        t = a.reshape(-1, P, F)                    # [T, P, F]
        return np.ascontiguousarray(t.transpose(1, 0, 2).reshape(P, -1))

    in_maps = []
    for b in range(NCORES):
        m = {}
        wp = wpack.copy()
        wp[:, VL_C] = float(vl[b])
        m["wpack"] = wp
        m["qpk"] = pack_tiles(qs[b])
        m["kpk"] = pack_tiles(ks[b])
        va = np.ones((KT, P, F + 1), np.float32)
        va[:, :, :F] = vs[b].reshape(KT, P, F)
        m["vaug"] = np.ascontiguousarray(va.transpose(1, 0, 2).reshape(P, -1))
        in_maps.append(m)
    return in_maps


def kernel(queries, keys, values, valid_lens, Wq, bq, Wk, bk, w_s, b_s):
    global LAST_RESULT
    nc = get_nc()
    in_maps = make_in_maps(queries, keys, values, valid_lens, Wq, bq, Wk, bk, w_s, b_s)
    res = run_bass_kernel_spmd(nc, in_maps, list(range(NCORES)), trace=TRACE)
    LAST_RESULT = res
    out = np.stack([res.results[b]["out"] for b in range(NCORES)], axis=0)
    # [P, T*F] -> [LQ, F]
    out = out.reshape(B, P, QT, F).transpose(0, 2, 1, 3).reshape(B, LQ, F)
    return np.ascontiguousarray(out)
